# revision 16
# baseline (speedup 1.0000x reference)
"""Trainium2 Bass kernel for an AttentionBlock with a single KV token.

Math: with kv_len == 1 the softmax over the key axis is identically 1.0,
so the attention output for every query position equals v, and the
LayerNorm / q-projection never influence the output:

    kv      = cond_emb @ kv_w.T + kv_b          # (b, 2c)
    v_in    = kv[:, c:]                         # (b, c)
    v_full  = v_in @ wv.T + bv                  # (b, c)   wv = in_proj_w[2c:]
    av      = v_full @ out_w.T + out_b          # (b, c)
    y       = x + av[:, :, None, None]          # (b, c, h, w)

This is a tiny per-batch matmul chain plus one huge memory-bound
broadcast add.  Sharding: data-parallel over batch (8 batches/core),
weights replicated (host pre-transposed into matmul layouts).
"""

import numpy as np

import concourse.bacc as bacc
import concourse.mybir as mybir
from concourse.bass_utils import run_bass_kernel_spmd
from concourse.tile import TileContext

B, C, H, W = 64, 256, 64, 64
EMB = 512
HWD = H * W               # 4096
NCORES = 8
BS = B // NCORES          # 8 batches per core
ROWS = BS * C             # 2048 rows of length HW per core
NT = ROWS // 128          # 16 tiles of [128, 4096]
F32 = mybir.dt.float32

_CACHE = {}


# Column offsets inside the packed consts tensor [128, CONST_COLS]:
#   cond:  [p, e*8 + b]        = cond_emb[b, 128e + p]           (32 cols)
#   kvw:   [p, e*256 + j]      = kv_w[256 + j, 128e + p]         (1024 cols)
#   wv:    [p, i*256 + j]      = in_proj_w[512 + j, 128i + p]    (512 cols)
#   outw:  [p, j*256 + c]      = out_w[c, 128j + p]              (512 cols)
#   bias:  [p, u*3 + k]; k=0: kv_b[256+u*128+p],
#          k=1: in_proj_b[512+u*128+p], k=2: out_b[u*128+p]      (6 cols)
COND_O = 0
KVW_O = COND_O + 4 * BS
WV_O = KVW_O + 4 * C
OUTW_O = WV_O + 2 * C
BIAS_O = OUTW_O + 2 * C
CONST_COLS = BIAS_O + 6


def _build_nc():
    nc = bacc.Bacc("TRN2", target_bir_lowering=False, debug=False)

    x_d = nc.dram_tensor("x", [ROWS, HWD], F32, kind="ExternalInput").ap()
    consts_d = nc.dram_tensor("consts", [128, CONST_COLS], F32, kind="ExternalInput").ap()
    y_d = nc.dram_tensor("y", [ROWS, HWD], F32, kind="ExternalOutput").ap()

    with TileContext(nc) as tc:
        with (
            tc.tile_pool(name="const", bufs=1) as cpool,
            tc.tile_pool(name="psum", bufs=2, space="PSUM") as ppool,
            tc.tile_pool(name="small", bufs=2) as spool,
            tc.tile_pool(name="xio", bufs=9) as xpool,
            tc.tile_pool(name="xhalf", bufs=4) as hpool,
        ):
            csb = cpool.tile([128, CONST_COLS], F32, tag="consts")
            # Head of the scalar HWDGE ring: stores don't exist for the
            # first ~14us, so this costs nothing and keeps the sync ring
            # free to start streaming x immediately.
            nc.scalar.dma_start(out=csb[:], in_=consts_d[:])
            cond_sb = csb[:, COND_O : COND_O + 4 * BS]
            kvw_sb = csb[:, KVW_O : KVW_O + 4 * C]
            wv_sb = csb[:, WV_O : WV_O + 2 * C]
            outw_sb = csb[:, OUTW_O : OUTW_O + 2 * C]
            bias_sb = csb[:, BIAS_O : BIAS_O + 6]

            # v_inT[u][p, b] = kv[b, 256 + u*128 + p]
            vin_sb = [spool.tile([128, BS], F32, tag=f"vin{u}", name=f"vin{u}") for u in range(2)]
            for u in range(2):
                pv = ppool.tile([128, BS], F32)
                for e in range(4):
                    nc.tensor.matmul(
                        out=pv[:],
                        lhsT=kvw_sb[:, e * C + u * 128 : e * C + u * 128 + 128],
                        rhs=cond_sb[:, e * BS : (e + 1) * BS],
                        start=(e == 0),
                        stop=(e == 3),
                    )
                nc.vector.tensor_scalar_add(
                    out=vin_sb[u][:], in0=pv[:], scalar1=bias_sb[:, 0 + u * 3 : 1 + u * 3]
                )

            # v_fullT[u][p, b] = v_full[b, u*128 + p]
            vf_sb = [spool.tile([128, BS], F32, tag=f"vf{u}", name=f"vf{u}") for u in range(2)]
            for u in range(2):
                pv = ppool.tile([128, BS], F32)
                for i in range(2):
                    nc.tensor.matmul(
                        out=pv[:],
                        lhsT=wv_sb[:, i * C + u * 128 : i * C + u * 128 + 128],
                        rhs=vin_sb[i][:],
                        start=(i == 0),
                        stop=(i == 1),
                    )
                nc.vector.tensor_scalar_add(
                    out=vf_sb[u][:], in0=pv[:], scalar1=bias_sb[:, 1 + u * 3 : 2 + u * 3]
                )

            # avT[u][p, b] = av[b, u*128 + p]
            av_sb = [spool.tile([128, BS], F32, tag=f"av{u}", name=f"av{u}") for u in range(2)]
            for u in range(2):
                pv = ppool.tile([128, BS], F32)
                for j in range(2):
                    nc.tensor.matmul(
                        out=pv[:],
                        lhsT=outw_sb[:, j * C + u * 128 : j * C + u * 128 + 128],
                        rhs=vf_sb[j][:],
                        start=(j == 0),
                        stop=(j == 1),
                    )
                nc.vector.tensor_scalar_add(
                    out=av_sb[u][:], in0=pv[:], scalar1=bias_sb[:, 2 + u * 3 : 3 + u * 3]
                )

            # Stream x: row r = b*256 + c ; tile t covers rows [128t, 128t+128)
            # -> batch b = t//2, channel c = (t%2)*128 + p, scalar = av_sb[t%2][p, t//2]
            def add_store(tile_ap, dram_rows, av_ap, store_eng):
                # Broadcast-add on DVE (2x mode, ~2.8us/full tile) in-place.
                nc.vector.tensor_scalar_add(out=tile_ap, in0=tile_ap, scalar1=av_ap)
                store_eng.dma_start(out=dram_rows, in_=tile_ap)

            # Stores default to the scalar HWDGE ring; the tail stores
            # alternate onto the sync ring (empty once loads finish) so the
            # stores-only end phase runs dual-row at full DMA rate.
            HH = HWD // 2
            SYNC_TAIL = {12}  # stores deferred to the sync ring tail
            tail_stores = []
            for t in range(NT):
                u, b = t % 2, t // 2
                av_ap = av_sb[u][:, b : b + 1]
                rows = slice(t * 128, (t + 1) * 128)
                if t == 0:
                    # Quarter the first tile: small first DMAs ramp the SDMA
                    # engines faster and the first store issues sooner.
                    QQ = HWD // 4
                    for h in range(4):
                        quar = hpool.tile([128, QQ], F32, tag="xq", name=f"xq{h}")
                        cols = slice(h * QQ, (h + 1) * QQ)
                        nc.sync.dma_start(out=quar[:], in_=x_d[rows, cols])
                        add_store(quar[:], y_d[rows, cols], av_ap, nc.scalar)
                elif t == NT - 1:
                    # Split the last tile: short add+store pipeline tail
                    # after the final load.
                    for h in range(2):
                        half = hpool.tile([128, HH], F32, tag="xh", name=f"xh{t}_{h}")
                        cols = slice(h * HH, (h + 1) * HH)
                        nc.sync.dma_start(out=half[:], in_=x_d[rows, cols])
                        if h == 1:
                            nc.vector.tensor_scalar_add(
                                out=half[:], in0=half[:], scalar1=av_ap
                            )
                            tail_stores.append((y_d[rows, cols], half[:]))
                        else:
                            add_store(half[:], y_d[rows, cols], av_ap, nc.scalar)
                elif t == 14:
                    # Split this store across the rings: first half to the
                    # scalar ring now, second half to the sync-ring tail.
                    tile = xpool.tile([128, HWD], F32, tag="xt")
                    nc.sync.dma_start(out=tile[:], in_=x_d[rows, :])
                    nc.vector.tensor_scalar_add(out=tile[:], in0=tile[:], scalar1=av_ap)
                    nc.scalar.dma_start(out=y_d[rows, 0:HH], in_=tile[:, 0:HH])
                    tail_stores.append((y_d[rows, HH:], tile[:, HH:]))
                else:
                    tile = xpool.tile([128, HWD], F32, tag="xt")
                    nc.sync.dma_start(out=tile[:], in_=x_d[rows, :])
                    if t in SYNC_TAIL:
                        nc.vector.tensor_scalar_add(
                            out=tile[:], in0=tile[:], scalar1=av_ap
                        )
                        tail_stores.append((y_d[rows, :], tile[:]))
                    else:
                        add_store(tile[:], y_d[rows, :], av_ap, nc.scalar)
            # Issued after every load in program order -> they sit at the end
            # of the sync ring FIFO and never block a load.
            for dst, src in tail_stores:
                nc.sync.dma_start(out=dst, in_=src)

    nc.compile()
    return nc


def _prep_consts(in_proj_w, in_proj_b, out_w, out_b, kv_w, kv_b):
    c = C
    base = np.empty((128, CONST_COLS), np.float32)
    base[:, KVW_O : KVW_O + 4 * c] = (
        kv_w[c : 2 * c, :].T.reshape(4, 128, c).transpose(1, 0, 2).reshape(128, 4 * c)
    )
    base[:, WV_O : WV_O + 2 * c] = (
        in_proj_w[2 * c :, :].T.reshape(2, 128, c).transpose(1, 0, 2).reshape(128, 2 * c)
    )
    base[:, OUTW_O : OUTW_O + 2 * c] = (
        out_w.T.reshape(2, 128, c).transpose(1, 0, 2).reshape(128, 2 * c)
    )
    for u in range(2):
        base[:, BIAS_O + u * 3 + 0] = kv_b[c + u * 128 : c + (u + 1) * 128]
        base[:, BIAS_O + u * 3 + 1] = in_proj_b[2 * c + u * 128 : 2 * c + (u + 1) * 128]
        base[:, BIAS_O + u * 3 + 2] = out_b[u * 128 : (u + 1) * 128]
    return base


def make_in_maps(x, cond_emb, in_proj_w, in_proj_b, out_w, out_b, kv_w, kv_b):
    base = _prep_consts(in_proj_w, in_proj_b, out_w, out_b, kv_w, kv_b)
    in_maps = []
    for r in range(NCORES):
        xs = np.ascontiguousarray(
            x[r * BS : (r + 1) * BS].reshape(ROWS, HWD), dtype=np.float32
        )
        consts = base.copy()
        consts[:, COND_O : COND_O + 4 * BS] = (
            cond_emb[r * BS : (r + 1) * BS]
            .T.reshape(4, 128, BS)
            .transpose(1, 0, 2)
            .reshape(128, 4 * BS)
        )
        in_maps.append({"x": xs, "consts": consts})
    return in_maps


def get_nc():
    if "nc" not in _CACHE:
        _CACHE["nc"] = _build_nc()
    return _CACHE["nc"]


def kernel(x, cond_emb, ln_gamma, ln_beta, in_proj_w, in_proj_b, out_w, out_b, kv_w, kv_b):
    x = np.asarray(x, dtype=np.float32)
    nc = get_nc()
    in_maps = make_in_maps(
        x,
        np.asarray(cond_emb, np.float32),
        np.asarray(in_proj_w, np.float32),
        np.asarray(in_proj_b, np.float32),
        np.asarray(out_w, np.float32),
        np.asarray(out_b, np.float32),
        np.asarray(kv_w, np.float32),
        np.asarray(kv_b, np.float32),
    )
    res = run_bass_kernel_spmd(nc, in_maps, core_ids=list(range(NCORES)))
    y = np.empty((B, C, H, W), np.float32)
    for r in range(NCORES):
        y[r * BS : (r + 1) * BS] = res.results[r]["y"].reshape(BS, C, H, W)
    return y
